# revision 16
# baseline (speedup 1.0000x reference)
"""DeepPoseGCN on 8 Trainium2 NeuronCores (Bass/Tile, SPMD).

Self-contained: host-side graph preprocessing (index math only) + one Bass
program shared by all 8 cores (per-core content via input arrays).

- nodes partitioned by dst across 8 cores (12500 each)
- per-layer htilde tables exchanged via AllGather (bf16 [*,128] rows)
- aggregation: dma_gather (256B rows, int16 bank-local idx, 4 banks)
  -> PE stage-1 blockdiag-U4 matmuls (group-of-4 partial sums)
  -> PE stage-2 matmuls (lhsT = C4 column, rhs = on-device one-hot A2)
  into feature-major PSUM node-windows
- pointwise (folded BN, relu, residual, 1/sqrt(deg) scales) feature-major
- mean-pool via one-hot matmuls, MLP head + log_softmax on every core
"""
import numpy as np

N = 100000
E = 3200000
G = 1024
NC = 8
NSH = N // NC            # 12500
SHROWS = NSH + 2         # 12502
BANKW = 2 * SHROWS       # 25004
ZLOC = NSH
NTILE = 128
NT = (NSH + NTILE - 1) // NTILE      # 98
NPAD = NT * NTILE                    # 12544
SBT = 16
NSB = (NT + SBT - 1) // SBT          # 7
GRP = 4
REGALN = 1024
SBALN = 3072
CHUNK = 1024
D = 64
NBANK = 4
GRAN = 3072
PSTACK = 96
GPL = 256
NWIN = 512
BN_EPS = 1e-5
f32 = np.float32


def _col_groups(wincol):
    g, t = wincol // 8, wincol % 8
    j = np.arange(PSTACK) // 32
    m = np.arange(PSTACK) % 32
    return 768 * g + 256 * j + 32 * t + m


def _prep(src, dst, batch):
    s_all = np.concatenate([src.astype(np.int64), np.arange(N, dtype=np.int64)])
    d_all = np.concatenate([dst.astype(np.int64), np.arange(N, dtype=np.int64)])
    deg = np.bincount(d_all, minlength=N).astype(f32)

    per_core = []
    cnt_nb = np.zeros((NC, NSH, NBANK), np.int64)
    for c in range(NC):
        m = (d_all // NSH) == c
        sc = s_all[m]
        dc = d_all[m] - c * NSH
        bk = (sc // NSH) // 2
        sl = ((sc // NSH) % 2) * SHROWS + (sc % NSH)
        order = np.lexsort((sl, bk, dc))
        dc, bk, sl = dc[order], bk[order], sl[order]
        np.add.at(cnt_nb[c], (dc, bk), 1)
        per_core.append((dc, bk, sl))

    pad_nb = ((cnt_nb + GRP - 1) // GRP) * GRP
    tile_of = np.arange(NSH) // NTILE
    run_sums = np.zeros((NC, NT, NBANK), np.int64)
    for c in range(NC):
        for b in range(NBANK):
            run_sums[c, :, b] = np.bincount(
                tile_of, weights=pad_nb[c, :, b], minlength=NT).astype(np.int64)
    R = run_sums.max(0)

    REG = np.zeros((NSB, NBANK), np.int64)
    for sb in range(NSB):
        REG[sb] = R[sb * SBT:min((sb + 1) * SBT, NT)].sum(0)
    REG = ((REG + REGALN - 1) // REGALN) * REGALN
    for sb in range(NSB):
        REG[sb, NBANK - 1] += (-REG[sb].sum()) % SBALN

    reg_off = np.zeros((NSB, NBANK), np.int64)
    pos = 0
    for sb in range(NSB):
        for b in range(NBANK):
            reg_off[sb, b] = pos
            pos += REG[sb, b]
    EP = int(pos)
    run_off = np.zeros((NT, NBANK), np.int64)
    for sb in range(NSB):
        for b in range(NBANK):
            p = reg_off[sb, b]
            for t in range(sb * SBT, min((sb + 1) * SBT, NT)):
                run_off[t, b] = p
                p += R[t, b]

    sb_estart = np.zeros(NSB + 1, np.int64)
    for sb in range(NSB):
        sb_estart[sb + 1] = sb_estart[sb] + REG[sb].sum()
    sb_gstart = sb_estart // GRP

    calls = []          # (sb, bank, p0, ne)
    for sb in range(NSB):
        for b in range(NBANK):
            p0, rem = int(reg_off[sb, b]), int(REG[sb, b])
            while rem > 0:
                ne = min(CHUNK, rem)
                calls.append((sb, b, p0, ne))
                p0 += ne
                rem -= ne

    pos2tile = np.full(EP, -1, np.int64)
    for t in range(NT):
        for b in range(NBANK):
            pos2tile[run_off[t, b]:run_off[t, b] + R[t, b]] = t
    pos2reg = np.zeros(EP, np.int64)
    for sb in range(NSB):
        for b in range(NBANK):
            pos2reg[reg_off[sb, b]:reg_off[sb, b] + REG[sb, b]] = sb * NBANK + b

    cores = []
    for c in range(NC):
        dc, bk, sl = per_core[c]
        pstart = np.zeros((NSH, NBANK), np.int64)
        for b in range(NBANK):
            padc = pad_nb[c, :, b]
            cs = np.cumsum(padc) - padc
            tb = np.cumsum(np.bincount(tile_of, weights=padc,
                                       minlength=NT)).astype(np.int64)
            tile_base = np.zeros(NSH, np.int64)
            tile_base[NTILE:] = np.repeat(tb[:-1], NTILE)[:NSH - NTILE]
            pstart[:, b] = run_off[tile_of, b] + (cs - tile_base)
        gidx = np.full(EP, ZLOC, np.int32)
        segkey = dc * NBANK + bk
        chg = np.ones(len(segkey), bool)
        chg[1:] = segkey[1:] != segkey[:-1]
        seg_starts = np.nonzero(chg)[0]
        seg_counts = np.diff(np.append(seg_starts, len(segkey)))
        rank = np.arange(len(dc)) - np.repeat(seg_starts, seg_counts)
        gidx[pstart[dc, bk] + rank] = sl
        gq = EP // GRP
        gnode = np.full(gq, -1, np.int64)
        gs = pstart // GRP
        glen = pad_nb[c] // GRP
        nn, bb = np.nonzero(glen > 0)
        starts, lens = gs[nn, bb], glen[nn, bb]
        fill = np.repeat(starts, lens) + (
            np.arange(lens.sum()) - np.repeat(np.cumsum(lens) - lens, lens))
        gnode[fill] = np.repeat(nn, lens)
        w = gidx.astype(np.int16).reshape(-1, 16).T
        cores.append(dict(gnode=gnode, gidx_w=np.tile(w, (8, 1))))

    # stage-2 segments: per C4 column, node-coverage slabs of <=128 nodes.
    # A2 rows for nodes outside a seg's slab are -1 -> zero one-hot rows, so
    # every matmul uses the full [0:PSTACK] partition range (PE base 0).
    segs = []       # sb, col, n0 (slab base node, core-local)
    seg_nodes = []  # per seg: per-core row->node array [NC, PSTACK]
    for sb in range(NSB):
        ncols = int((sb_gstart[sb + 1] - sb_gstart[sb]) // PSTACK)
        for col in range(ncols):
            gg = sb_gstart[sb] + _col_groups(col)
            nod = np.full((NC, PSTACK), -1, np.int64)
            for c in range(NC):
                nod[c] = cores[c]["gnode"][gg]
            valid = nod >= 0
            if not valid.any():
                continue
            rest = valid.copy()
            while rest.any():
                n0 = int(nod[rest].min())
                take = rest & (nod < n0 + 128)
                segs.append(dict(sb=sb, col=col, n0=n0))
                seg_nodes.append(np.where(take, nod, -1))
                rest = rest & ~take
    NSEG = len(segs)

    gno = np.full((NC, 128, NSEG), -1, np.int16)
    for i, sg in enumerate(segs):
        nod = seg_nodes[i]
        for c in range(NC):
            off = nod[c] - sg["n0"]
            ok = nod[c] >= 0
            gno[c, :PSTACK, i] = np.where(ok, off, -1).astype(np.int16)

    mms = []
    for i, sg in enumerate(segs):
        n0 = sg["n0"]
        lo = n0
        while lo < min(n0 + 128, NSH):
            w = lo // NWIN
            hi = min(n0 + 128, (w + 1) * NWIN, NSH)
            mms.append(dict(seg=i, sb=sg["sb"], col=sg["col"], win=w,
                            slot=lo - n0, n=hi - lo, poff=lo - w * NWIN))
            lo = hi
    first_mm, last_mm = {}, {}
    for k, mm in enumerate(mms):
        first_mm.setdefault(mm["win"], k)
        last_mm[mm["win"]] = k
    for k, mm in enumerate(mms):
        mm["start"] = first_mm[mm["win"]] == k
        mm["stop"] = last_mm[mm["win"]] == k

    gfirst = np.zeros(NC, np.int64)
    bpool = np.zeros((NC, NT, NTILE, GPL), f32)
    for c in range(NC):
        bc = batch[c * NSH:(c + 1) * NSH].astype(np.int64)
        gfirst[c] = bc.min()
        assert bc.max() - bc.min() + 1 <= GPL
        loc = bc - gfirst[c]
        n = np.arange(NSH)
        bpool[c, n // NTILE, n % NTILE, loc] = 1.0
    cb = np.zeros((NC * GPL, G), f32)
    for c in range(NC):
        gcnt = int(batch[(c + 1) * NSH - 1]) - int(gfirst[c]) + 1
        l = np.arange(gcnt)
        cb[c * GPL + l, gfirst[c] + l] = 1.0
    cnt = np.bincount(batch.astype(np.int64), minlength=G).astype(f32)
    invcnt = (1.0 / np.maximum(cnt, 1.0)).astype(f32)

    meta = dict(EP=EP, calls=calls, sb_gstart=sb_gstart, segs=segs, mms=mms,
                NSEG=NSEG)
    shared = dict(deg=deg, invcnt=invcnt, cb=cb)
    for c in range(NC):
        cores[c]["bpool"] = bpool[c]
        cores[c]["gno"] = gno[c]
        del cores[c]["gnode"]
    return cores, meta, shared


_CACHE = {}
_LAST_DEBUG = {}


def _build(meta):
    import concourse.bacc as bacc
    import concourse.mybir as mybir
    from concourse.tile import TileContext

    dt = mybir.dt
    AF = mybir.ActivationFunctionType
    OP = mybir.AluOpType
    nc = bacc.Bacc("TRN2", target_bir_lowering=False, debug=False,
                   num_devices=NC)
    EP = meta["EP"]
    NSEG = meta["NSEG"]
    sb_gstart = meta["sb_gstart"]
    calls = meta["calls"]
    segs = meta["segs"]
    mms = meta["mms"]
    WMAX = max(int((sb_gstart[sb + 1] - sb_gstart[sb]) // PSTACK)
               for sb in range(NSB))

    def din(name, shape, d=dt.float32):
        return nc.dram_tensor(name, shape, d, kind="ExternalInput").ap()

    x_fm = din("x_fm", [3, NPAD])
    isq_fm = din("isq_rep", [64, NPAD], dt.bfloat16)
    gidx_w = din("gidx_w", [128, EP // 16], dt.int16)
    gno_in = din("gno", [128, NSEG], dt.int16)
    iota8 = din("iota8", [128, 8 * 128], dt.int16)
    u4 = din("u4", [128, 32], dt.bfloat16)
    W1 = din("W1", [3, D])
    W2 = din("W2", [D, D])
    W3 = din("W3", [D, D])
    W4 = din("W4", [D, D])
    bnK = din("bnK", [4, 64, 1])
    bnC = din("bnC", [4, 64, 1])
    bpool_in = din("bpool", [NT, NTILE, GPL])
    cb_in = din("cb", [NC * GPL, G])
    invcnt_in = din("invcnt", [128, G // 128])
    lw1 = din("lw1", [D, 32])
    lw2 = din("lw2", [32, 2])
    lb1c = din("lb1c", [32, 1])
    lb2r = din("lb2r", [128, 2])
    ident = din("ident", [128, 128])

    out_ls = nc.dram_tensor("logits", [G, 2], dt.float32,
                            kind="ExternalOutput").ap()
    dbg_tab = nc.dram_tensor("dbg_tab", [256, 128], dt.float32,
                             kind="ExternalOutput").ap()
    dbg_h4 = nc.dram_tensor("dbg_h4", [64, 512], dt.float32,
                            kind="ExternalOutput").ap()
    dbg_pool = nc.dram_tensor("dbg_pool", [256, 64], dt.float32,
                              kind="ExternalOutput").ap()
    shard = nc.dram_tensor("shard", [SHROWS, 128], dt.bfloat16).ap()
    table = nc.dram_tensor("table", [NC * SHROWS, 128], dt.bfloat16,
                           addr_space="Shared").ap()
    pool_sh = nc.dram_tensor("pool_sh", [GPL, D], dt.float32).ap()
    pool_full = nc.dram_tensor("pool_full", [NC * GPL, D], dt.float32,
                               addr_space="Shared").ap()

    h_fm = nc.alloc_sbuf_tensor("h_fm", [64, NPAD], dt.float32).ap()
    tb_sb = nc.alloc_sbuf_tensor("tb_sb", [128, NT * 64], dt.bfloat16).ap()
    c4 = nc.alloc_sbuf_tensor("c4", [128, WMAX * 64], dt.bfloat16).ap()
    isq_sb = nc.alloc_sbuf_tensor("isq_sb", [64, NPAD], dt.bfloat16).ap()

    # group mms by sb; order by (window, seg) so at most ~2 psum windows
    # are live at a time (bank-major col order would keep all 4 alive)
    mms_by_sb = {}
    for k, mm in enumerate(mms):
        mms_by_sb.setdefault(mm["sb"], []).append(mm)
    for sb in mms_by_sb:
        mms_by_sb[sb].sort(key=lambda m: (m["win"], m["seg"], m["slot"]))
    # recompute start/stop in emission order
    firsts, lasts = {}, {}
    for sb in sorted(mms_by_sb):
        for m in mms_by_sb[sb]:
            firsts.setdefault(m["win"], id(m))
            lasts[m["win"]] = id(m)
    for sb in mms_by_sb:
        for m in mms_by_sb[sb]:
            m["start"] = firsts[m["win"]] == id(m)
            m["stop"] = lasts[m["win"]] == id(m)
    calls_by_sb = {}
    for (sb, b, p0, ne) in calls:
        calls_by_sb.setdefault(sb, []).append((b, p0, ne))

    with TileContext(nc) as tc:
        with tc.tile_pool(name="const", bufs=1) as cpool, \
             tc.tile_pool(name="work", bufs=3) as wpool, \
             tc.tile_pool(name="a2p", bufs=2) as a2pool, \
             tc.tile_pool(name="msgp", bufs=2) as msgpool:

            iota_t = cpool.tile([128, 8 * 128], dt.int16, tag="iota")
            nc.sync.dma_start(out=iota_t[:], in_=iota8[:])
            u4_t = cpool.tile([128, 32], dt.bfloat16, tag="u4")
            nc.sync.dma_start(out=u4_t[:], in_=u4[:])
            gno_t = cpool.tile([128, NSEG], dt.int16, tag="gno")
            nc.sync.dma_start(out=gno_t[:], in_=gno_in[:])
            id_t = cpool.tile([128, 128], dt.float32, tag="ident")
            nc.sync.dma_start(out=id_t[:], in_=ident[:])
            w1_t = cpool.tile([3, D], dt.float32, tag="w1")
            nc.sync.dma_start(out=w1_t[:], in_=W1[:])
            w2_t = cpool.tile([D, D], dt.float32, tag="w2")
            nc.sync.dma_start(out=w2_t[:], in_=W2[:])
            w3_t = cpool.tile([D, D], dt.float32, tag="w3")
            nc.sync.dma_start(out=w3_t[:], in_=W3[:])
            w4_t = cpool.tile([D, D], dt.float32, tag="w4")
            nc.sync.dma_start(out=w4_t[:], in_=W4[:])
            w_next = [w2_t, w3_t, w4_t]
            bnK_t = cpool.tile([64, 4], dt.float32, tag="bnK")
            nc.sync.dma_start(out=bnK_t[:], in_=bnK.rearrange("l p o -> p (l o)"))
            bnC_t = cpool.tile([64, 4], dt.float32, tag="bnC")
            nc.sync.dma_start(out=bnC_t[:], in_=bnC.rearrange("l p o -> p (l o)"))
            nc.sync.dma_start(out=isq_sb[:], in_=isq_fm[:])
            zpad = cpool.tile([128, 128], dt.bfloat16, tag="zpad")
            nc.vector.memset(zpad[:], 0.0)
            psA_cm = tc.tile_pool(name="psA", bufs=2, space="PSUM")
            psA = psA_cm.__enter__()
            psW_cm = tc.tile_pool(name="psW", bufs=2, space="PSUM")
            psW = psW_cm.__enter__()
            psM_cm = tc.tile_pool(name="psM", bufs=2, space="PSUM")
            psM = psM_cm.__enter__()

            # zero pad columns 64:128 of shard + pad rows, once
            for blk in range((SHROWS + 127) // 128):
                r0 = blk * 128
                r1 = min(r0 + 128, SHROWS)
                nc.sync.dma_start(out=shard[r0:r1, 64:128],
                                  in_=zpad[:r1 - r0, :64])
            nc.sync.dma_start(out=shard[NSH:SHROWS, 0:64], in_=zpad[:2, :64])

            nc.vector.memset(h_fm[:], 0.0)

            def build_table(wt, rhs_of_win):
                """table = ((rhs @ wt) * isq) per 512-window, transposed into
                tb_sb node-major bf16, then shard DMA + allgather."""
                for w in range(NPAD // NWIN):
                    ps = psW.tile([64, NWIN], dt.float32, tag="wmm")
                    nc.tensor.matmul(out=ps[:], lhsT=wt,
                                     rhs=rhs_of_win(w),
                                     start=True, stop=True)
                    tmp = wpool.tile([64, NWIN], dt.float32, tag="pw")
                    nc.vector.tensor_tensor(
                        out=tmp[:], in0=ps[:],
                        in1=isq_sb[:, w * NWIN:(w + 1) * NWIN], op=OP.mult)
                    for q in range(NWIN // 128):
                        t = w * (NWIN // 128) + q
                        pt = psM.tile([128, 64], dt.float32, tag="ptr")
                        nc.tensor.transpose(
                            out=pt[:], in_=tmp[:, q * 128:(q + 1) * 128],
                            identity=id_t[:64, :64])
                        nc.scalar.activation(
                            tb_sb[:, t * 64:(t + 1) * 64], pt[:], AF.Copy)
                tbv = tb_sb.rearrange("p (t d) -> p t d", d=64)
                nc.sync.dma_start(
                    out=shard[0:12416, 0:64].rearrange(
                        "(t p) d -> p t d", p=128),
                    in_=tbv[:, 0:97, :])
                nc.sync.dma_start(out=shard[12416:NSH, 0:64],
                                  in_=tbv[0:84, 97, :])
                nc.gpsimd.collective_compute(
                    "AllGather", OP.bypass,
                    replica_groups=[list(range(NC))],
                    ins=[shard[:]], outs=[table[:]])

            # layer-1 table from x (streamed in 512-col chunks)
            def x_rhs(w):
                xt = wpool.tile([3, NWIN], dt.float32, tag="xw")
                nc.sync.dma_start(out=xt[:],
                                  in_=x_fm[:, w * NWIN:(w + 1) * NWIN])
                return xt[:]
            build_table(w1_t[:], x_rhs)

            for tb in range(2):
                tt = wpool.tile([128, 128], dt.float32, tag="dbgt")
                nc.gpsimd.dma_start(out=tt[:],
                                    in_=table[tb * 128:(tb + 1) * 128, :])
                nc.sync.dma_start(out=dbg_tab[tb * 128:(tb + 1) * 128, :],
                                  in_=tt[:])
            c4v = c4.rearrange("p (c d) -> p c d", d=64)

            for li in range(4):
                win_ps = {}
                for sb in range(NSB):
                    sb_g0 = int(sb_gstart[sb])
                    # gather + stage-1
                    ps1_cur = None
                    for (bk, p0, ne) in calls_by_sb[sb]:
                        idx_t = wpool.tile([128, CHUNK // 16], dt.int16,
                                           tag="idx")
                        nc.sync.dma_start(
                            out=idx_t[:, :ne // 16],
                            in_=gidx_w[:, p0 // 16:(p0 + ne) // 16])
                        msg = msgpool.tile([128, CHUNK // 128, 128],
                                           dt.bfloat16, tag="msg")
                        nc.gpsimd.dma_gather(
                            out_ap=msg[:, :ne // 128, :],
                            in_ap=table[bk * BANKW:(bk + 1) * BANKW, :],
                            idxs_ap=idx_t[:, :ne // 16],
                            num_idxs=ne, num_idxs_reg=ne, elem_size=128)
                        for q in range(ne // 1024):
                            gpos = p0 + q * 1024
                            jj = (gpos % GRAN) // 1024
                            if jj == 0:
                                ps1_cur = psA.tile([PSTACK, 512], dt.float32,
                                                   tag="s1")
                            nc.tensor.matmul(
                                out=ps1_cur[32 * jj:32 * (jj + 1), :],
                                lhsT=u4_t[:],
                                rhs=msg[:, q * 8:(q + 1) * 8, 0:64],
                                start=True, stop=True, skip_group_check=True)
                            if (gpos + 1024) % GRAN == 0:
                                colbase = (gpos + 1024 - GRAN) // GRAN * 8 \
                                    - sb_g0 // PSTACK
                                nc.scalar.activation(
                                    c4v[0:PSTACK, colbase:colbase + 8, :],
                                    ps1_cur.rearrange("p (c d) -> p c d", d=64),
                                    AF.Copy)
                    # stage-2
                    cur_batch = -1
                    a2t = None
                    for mm in mms_by_sb.get(sb, []):
                        sgi = mm["seg"]
                        bat = sgi // 8
                        if bat != cur_batch:
                            cur_batch = bat
                            a2t = a2pool.tile([128, 8, 128], dt.bfloat16,
                                              tag="a2")
                            nb = min(8, NSEG - bat * 8)
                            nc.vector.tensor_tensor(
                                out=a2t[:, :nb, :],
                                in0=iota_t.rearrange(
                                    "p (a b) -> p a b", b=128)[:, :nb, :],
                                in1=gno_t[:, bat * 8:bat * 8 + nb, None]
                                .to_broadcast([128, nb, 128]),
                                op=OP.is_equal)
                        w = mm["win"]
                        if mm["start"]:
                            wtile = psW.tile([64, NWIN], dt.float32, tag="s2", name=f"s2w")
                            win_ps[w] = wtile
                            nc.vector.memset(win_ps[w][:], 0.0)
                        nc.tensor.matmul(
                            out=win_ps[w][:, mm["poff"]:mm["poff"] + mm["n"]],
                            lhsT=c4v[0:PSTACK, mm["col"], :],
                            rhs=a2t[0:PSTACK, sgi % 8,
                                    mm["slot"]:mm["slot"] + mm["n"]],
                            start=mm["start"], stop=mm["stop"],
                            skip_group_check=True)
                        if mm["stop"]:
                            ps = win_ps.pop(w)
                            w0 = w * NWIN
                            wn = min(NWIN, NPAD - w0)
                            t1 = wpool.tile([64, NWIN], dt.float32, tag="pw")
                            nc.vector.tensor_tensor(
                                out=t1[:, :wn], in0=ps[:, :wn],
                                in1=isq_sb[:, w0:w0 + wn], op=OP.mult)
                            fn = AF.Relu if li < 3 else AF.Identity
                            t2 = wpool.tile([64, NWIN], dt.float32, tag="pw2")
                            nc.scalar.activation(
                                t2[:, :wn], t1[:, :wn], fn,
                                bias=bnC_t[:, li:li + 1],
                                scale=bnK_t[:, li:li + 1])
                            if 0 < li < 3:
                                nc.vector.tensor_tensor(
                                    out=h_fm[:, w0:w0 + wn],
                                    in0=t2[:, :wn],
                                    in1=h_fm[:, w0:w0 + wn], op=OP.add)
                            else:
                                nc.vector.tensor_copy(
                                    h_fm[:, w0:w0 + wn], t2[:, :wn])
                if li < 3:
                    build_table(w_next[li][:],
                                lambda w: h_fm[:, w * NWIN:(w + 1) * NWIN])

            psM_cm.__exit__(None, None, None)
            psW_cm.__exit__(None, None, None)
            psA_cm.__exit__(None, None, None)
            psH_cm = tc.tile_pool(name="psH", bufs=1, space="PSUM")
            psH = psH_cm.__enter__()

            # pooling
            pp0 = psH.tile([128, 64], dt.float32, tag="pool0")
            pp1 = psH.tile([128, 64], dt.float32, tag="pool1")
            for t in range(NT):
                ps = psH.tile([128, 64], dt.float32, tag="h4t")
                nc.tensor.transpose(out=ps[:],
                                    in_=h_fm[:, t * 128:(t + 1) * 128],
                                    identity=id_t[:64, :64])
                h4t = wpool.tile([128, 64], dt.float32, tag="h4s")
                nc.vector.tensor_copy(h4t[:], ps[:])
                bp = wpool.tile([128, GPL], dt.float32, tag="bp")
                nc.sync.dma_start(out=bp[:], in_=bpool_in[t, :, :])
                nc.tensor.matmul(out=pp0[:], lhsT=bp[:, 0:128], rhs=h4t[:],
                                 start=(t == 0), stop=(t == NT - 1),
                                 skip_group_check=True)
                nc.tensor.matmul(out=pp1[:], lhsT=bp[:, 128:256], rhs=h4t[:],
                                 start=(t == 0), stop=(t == NT - 1),
                                 skip_group_check=True)
            pl = wpool.tile([128, 2, 64], dt.float32, tag="pl")
            nc.vector.tensor_copy(pl[:, 0, :], pp0[:])
            nc.vector.tensor_copy(pl[:, 1, :], pp1[:])
            nc.sync.dma_start(
                out=pool_sh.rearrange("(a p) d -> p a d", p=128), in_=pl[:])
            nc.gpsimd.collective_compute(
                "AllGather", OP.bypass, replica_groups=[list(range(NC))],
                ins=[pool_sh[:]], outs=[pool_full[:]])

            iv = cpool.tile([128, G // 128], dt.float32, tag="iv")
            nc.sync.dma_start(out=iv[:], in_=invcnt_in[:])
            lw1_t = cpool.tile([D, 32], dt.float32, tag="lw1")
            nc.sync.dma_start(out=lw1_t[:], in_=lw1[:])
            lw2_t = cpool.tile([32, 2], dt.float32, tag="lw2")
            nc.sync.dma_start(out=lw2_t[:], in_=lw2[:])
            lb1_t = cpool.tile([32, 1], dt.float32, tag="lb1")
            nc.sync.dma_start(out=lb1_t[:], in_=lb1c[:])
            lb2_t = cpool.tile([128, 2], dt.float32, tag="lb2")
            nc.sync.dma_start(out=lb2_t[:], in_=lb2r[:])

            dh = wpool.tile([64, 512], dt.float32, tag="dbgh")
            nc.vector.tensor_copy(dh[:], h_fm[:, 0:512])
            nc.sync.dma_start(out=dbg_h4[:], in_=dh[:])
            dp = wpool.tile([128, 2, 64], dt.float32, tag="dbgp")
            nc.sync.dma_start(out=dp[:],
                              in_=pool_full.rearrange(
                                  "(a p) d -> p a d", p=128)[:, 0:2, :])
            nc.sync.dma_start(out=dbg_pool.rearrange("(a p) d -> p a d", p=128),
                              in_=dp[:])
            NK = NC * GPL // 128
            for gt in range(G // 128):
                ps = psH.tile([128, 64], dt.float32, tag="comb")
                for kk in range(NK):
                    pf = wpool.tile([128, 64], dt.float32, tag="pf")
                    nc.sync.dma_start(
                        out=pf[:], in_=pool_full[kk * 128:(kk + 1) * 128, :])
                    cbt = wpool.tile([128, 128], dt.float32, tag="cbt")
                    nc.sync.dma_start(
                        out=cbt[:], in_=cb_in[kk * 128:(kk + 1) * 128,
                                             gt * 128:(gt + 1) * 128])
                    nc.tensor.matmul(out=ps[:], lhsT=cbt[:], rhs=pf[:],
                                     start=(kk == 0), stop=(kk == NK - 1),
                                     skip_group_check=True)
                pooled = wpool.tile([128, 64], dt.float32, tag="pooled")
                nc.scalar.activation(pooled[:], ps[:], AF.Copy,
                                     scale=iv[:, gt:gt + 1])
                pt = psH.tile([64, 128], dt.float32, tag="poolT")
                nc.tensor.transpose(out=pt[:], in_=pooled[:], identity=id_t[:])
                pts = wpool.tile([64, 128], dt.float32, tag="poolTs")
                nc.vector.tensor_copy(pts[:], pt[:])
                h1p = psH.tile([32, 128], dt.float32, tag="h1")
                nc.tensor.matmul(out=h1p[:], lhsT=lw1_t[:], rhs=pts[:],
                                 start=True, stop=True)
                h1s = wpool.tile([32, 128], dt.float32, tag="h1s")
                nc.scalar.activation(h1s[:], h1p[:], AF.Relu, bias=lb1_t[:])
                lgp = psH.tile([128, 2], dt.float32, tag="lg")
                nc.tensor.matmul(out=lgp[:], lhsT=h1s[:], rhs=lw2_t[:],
                                 start=True, stop=True)
                lg = wpool.tile([128, 2], dt.float32, tag="lgs")
                nc.vector.tensor_tensor(out=lg[:], in0=lgp[:], in1=lb2_t[:],
                                        op=OP.add)
                mx = wpool.tile([128, 1], dt.float32, tag="mx")
                nc.vector.tensor_reduce(out=mx[:], in_=lg[:],
                                        axis=mybir.AxisListType.X, op=OP.max)
                nmx = wpool.tile([128, 1], dt.float32, tag="nmx")
                nc.vector.tensor_scalar_mul(nmx[:], mx[:], -1.0)
                ex = wpool.tile([128, 2], dt.float32, tag="ex")
                nc.scalar.activation(ex[:], lg[:], AF.Exp, bias=nmx[:])
                sm = wpool.tile([128, 1], dt.float32, tag="sm")
                nc.vector.tensor_reduce(out=sm[:], in_=ex[:],
                                        axis=mybir.AxisListType.X, op=OP.add)
                lsm = wpool.tile([128, 1], dt.float32, tag="lsm")
                nc.scalar.activation(lsm[:], sm[:], AF.Ln)
                off = wpool.tile([128, 1], dt.float32, tag="off")
                nc.vector.tensor_tensor(out=off[:], in0=mx[:], in1=lsm[:],
                                        op=OP.add)
                res = wpool.tile([128, 2], dt.float32, tag="res")
                nc.vector.tensor_scalar(out=res[:], in0=lg[:],
                                        scalar1=off[:], scalar2=None,
                                        op0=OP.subtract)
                nc.sync.dma_start(out=out_ls[gt * 128:(gt + 1) * 128, :],
                                  in_=res[:])
            psH_cm.__exit__(None, None, None)

    nc.finalize()
    return nc


def _u4_np():
    import ml_dtypes
    u = np.zeros((128, 32), np.float32)
    for k in range(128):
        u[k, k // GRP] = 1.0
    return u.astype(ml_dtypes.bfloat16)


def kernel(**inputs):
    from concourse.bass_utils import run_bass_kernel_spmd

    src = np.asarray(inputs["src"])
    dst = np.asarray(inputs["dst"])
    batch = np.asarray(inputs["batch"])
    key = (src.tobytes(), dst.tobytes(), batch.tobytes())
    key = hash(key)
    if key in _CACHE:
        nc, cores, meta, shared = _CACHE[key]
    else:
        cores, meta, shared = _prep(src, dst, batch)
        nc = _build(meta)
        _CACHE[key] = (nc, cores, meta, shared)

    deg = shared["deg"]
    isq = (1.0 / np.sqrt(np.maximum(deg, 1.0))).astype(f32)
    x = np.asarray(inputs["x"], f32)

    bnKs, bnCs = [], []
    for li in range(3):
        g_ = np.asarray(inputs[f"g{li+1}"], f32)
        be = np.asarray(inputs[f"be{li+1}"], f32)
        m_ = np.asarray(inputs[f"m{li+1}"], f32)
        v_ = np.asarray(inputs[f"v{li+1}"], f32)
        b_ = np.asarray(inputs[f"b{li+1}"], f32)
        K = (g_ / np.sqrt(v_ + BN_EPS)).astype(f32)
        C = ((b_ - m_) * K + be).astype(f32)
        bnKs.append(K)
        bnCs.append(C)
    bnKs.append(np.ones(D, f32))
    bnCs.append(np.asarray(inputs["b4"], f32))
    bnK = np.stack(bnKs)[:, :, None]
    bnC = np.stack(bnCs)[:, :, None]

    iota8 = np.tile(np.arange(128, dtype=np.int16)[None, :],
                    (128, 8)).reshape(128, 8 * 128)
    ident = np.eye(128, dtype=f32)

    in_maps = []
    for c in range(NC):
        x_fmc = np.zeros((3, NPAD), f32)
        x_fmc[:, :NSH] = x[c * NSH:(c + 1) * NSH].T
        import ml_dtypes
        isq_rep = np.zeros((64, NPAD), ml_dtypes.bfloat16)
        isq_rep[:, :NSH] = isq[c * NSH:(c + 1) * NSH][None, :].astype(
            ml_dtypes.bfloat16)
        in_maps.append(dict(
            x_fm=x_fmc, isq_rep=isq_rep,
            gidx_w=cores[c]["gidx_w"].astype(np.int16),
            gno=cores[c]["gno"], iota8=iota8, u4=_u4_np(),
            W1=np.asarray(inputs["W1"], f32),
            W2=np.asarray(inputs["W2"], f32),
            W3=np.asarray(inputs["W3"], f32),
            W4=np.asarray(inputs["W4"], f32),
            bnK=bnK, bnC=bnC, bpool=cores[c]["bpool"], cb=shared["cb"],
            invcnt=np.ascontiguousarray(
                shared["invcnt"].reshape(G // 128, 128).T),
            lw1=np.asarray(inputs["lw1"], f32),
            lw2=np.asarray(inputs["lw2"], f32),
            lb1c=np.asarray(inputs["lb1"], f32)[:, None],
            lb2r=np.tile(np.asarray(inputs["lb2"], f32)[None, :], (128, 1)),
            ident=ident,
        ))
    res = run_bass_kernel_spmd(nc, in_maps, list(range(NC)))
    global _LAST_DEBUG
    _LAST_DEBUG = {k: v for k, v in res.results[0].items()}
    return res.results[0]["logits"].astype(np.float32)
